# revision 11
# baseline (speedup 1.0000x reference)
"""DisplaceChannel Trainium2 kernel (Bass/Tile, 8-core SPMD data-parallel).

out[b, c, y, x] = x[b, c, y - oy(g), x - ox(g)], zero outside, g = c // 8.
Offsets are (iw*16, ih*16) for ih,iw in [-3..3] minus (0,0): 48 groups of 8
channels.  Shard batch (16) across 8 cores (2 each).

Per-core schedule: one SBUF tile per ox-class (7 classes).  A class's
groups (6 or 7) sit at 16-partition blocks, each block = group (8 ch) x 2
local batch elems (b-major), so every partition of the tile shares the
same x-shift.  The y-shift is free: loads place the used source rows at
their destination row offset (contiguous DRAM block per (b, ch)).  The
x-shift is then a single full-tile 2D copy on DVE; zero borders come from
a full input-tile memset (GPSIMD) plus a border-column memset (DVE).
Stores write only the valid output rows - the runner pre-zeroes
ExternalOutput buffers.
"""

import sys

if "/opt/trn_rl_repo" not in sys.path:
    sys.path.insert(0, "/opt/trn_rl_repo")

import numpy as np

import concourse.bass as bass
import concourse.mybir as mybir
from concourse import bacc
from concourse.bass_utils import run_bass_kernel_spmd
from concourse.tile import TileContext

# ---- problem constants (hardcoded; must match the reference) ----
H, W = 64, 64
STRIDE = 16
B, C = 16, 384
CP = 8                      # channels per group
NUM_POS = 48                # groups
N_CORES = 8
B_LOC = B // N_CORES        # 2
HW = H * W                  # 4096


def _offsets():
    offs = []
    for ih in range(-3, 4):
        for iw in range(-3, 4):
            if ih == 0 and iw == 0:
                continue
            offs.append((iw * STRIDE, ih * STRIDE))  # (off_x, off_y)
    return offs


OFFS = _offsets()

# ox-class -> list of group ids (natural order)
CLASSES = [[g for g in range(NUM_POS) if OFFS[g][0] == (t - 3) * STRIDE]
           for t in range(7)]


def build_program():
    # Bacc (not plain Bass): its compile pipeline splits multi-sem waits
    # into EVSEM chains (TRN2 allows only one wait per instruction).
    nc = bacc.Bacc("TRN2")
    x = nc.dram_tensor("x", [B_LOC, C, H, W], mybir.dt.float32,
                       kind="ExternalInput")
    out = nc.dram_tensor("out", [B_LOC, C, H, W], mybir.dt.float32,
                         kind="ExternalOutput")

    with TileContext(nc) as tc:
        with tc.tile_pool(name="inp", bufs=3) as inp, \
             tc.tile_pool(name="outp", bufs=3) as outp:
            for t, groups in enumerate(CLASSES):
                ox = (t - 3) * STRIDE
                ncols = W - abs(ox)
                xsrc, xdst = max(0, -ox), max(0, ox)

                data = inp.tile([128, HW], mybir.dt.float32)
                # zero everything; loads then overwrite the used rows, so
                # unused border rows read back as zeros for the x-copy.
                nc.gpsimd.memset(data[:, :], 0.0)

                for j, g in enumerate(groups):
                    oy = OFFS[g][1]
                    nrows = H - abs(oy)
                    ysrc, ydst = max(0, -oy), max(0, oy)
                    src = x[:, CP * g:CP * (g + 1), ysrc:ysrc + nrows, :] \
                        .rearrange("b c h w -> b c (h w)")
                    dst = data[16 * j:16 * j + 16,
                               ydst * W:(ydst + nrows) * W]
                    nc.sync.dma_start(out=dst, in_=src)

                ot = outp.tile([128, HW], mybir.dt.float32)
                dv = data.rearrange("p (h w) -> p h w", w=W)
                ov = ot.rearrange("p (h w) -> p h w", w=W)
                # x-shift: one whole-tile 2D copy (same ox on every partition)
                nc.vector.tensor_copy(
                    out=ov[:, :, xdst:xdst + ncols],
                    in_=dv[:, :, xsrc:xsrc + ncols])
                if ox != 0:
                    # zero the |ox| border columns
                    if ox > 0:
                        nc.vector.memset(ov[:, :, 0:xdst], 0.0)
                    else:
                        nc.vector.memset(ov[:, :, ncols:W], 0.0)

                for j, g in enumerate(groups):
                    oy = OFFS[g][1]
                    nrows = H - abs(oy)
                    ydst = max(0, oy)
                    dstd = out[:, CP * g:CP * (g + 1), ydst:ydst + nrows, :] \
                        .rearrange("b c h w -> b c (h w)")
                    srcs = ot[16 * j:16 * j + 16,
                              ydst * W:(ydst + nrows) * W]
                    nc.scalar.dma_start(out=dstd, in_=srcs)
    return nc


_NC_CACHE = None


def _get_nc():
    global _NC_CACHE
    if _NC_CACHE is None:
        nc = build_program()
        if not nc.is_finalized():
            nc.finalize()
        _NC_CACHE = nc
    return _NC_CACHE


def _run(x, trace=False, **kw):
    x = np.ascontiguousarray(np.asarray(x), dtype=np.float32)
    assert x.shape == (B, C, H, W)
    nc = _get_nc()
    in_maps = [{"x": x[i * B_LOC:(i + 1) * B_LOC]} for i in range(N_CORES)]
    res = run_bass_kernel_spmd(nc, in_maps, list(range(N_CORES)),
                               trace=trace, **kw)
    outs = [res.results[i]["out"].reshape(B_LOC, C, H, W)
            for i in range(N_CORES)]
    return np.concatenate(outs, axis=0), res


def kernel(x):
    out, _ = _run(x, trace=False)
    return out
